# revision 5
# baseline (speedup 1.0000x reference)
"""Trainium2 Bass kernel for the gated-MLP-over-ring-buffer problem.

Reference computation (B=512, M=128, V=256, H=256, IN = M*V = 32768):
    mem    = roll(memory, 1, axis=1); mem[:, 0, :] = x        # [B, M, V]
    flat   = mem.reshape(B, IN)                                # [B, 32768]
    h      = tanh(flat @ W1 + b1) * sigmoid(flat @ Wg + bg)    # [B, 256]
    logits = h @ W2 + b2                                       # [B, 256]

Strategy (8 NeuronCores, one trn2 chip):
  - Contraction-shard the two big GEMMs: core c owns k-rows
    [4096c, 4096(c+1)) of W1/Wg and the matching slab of flat.T
    (host-prepared, transposed + packed so every DMA line is >=2KB
    contiguous per partition).
  - Each core computes partial P1.T / Pg.T = W.T @ flat.T  -> [H, B]
    accumulated over its 32 k-chunks in PSUM (bf16 operands, f32 acc).
  - Partials are downcast to bf16 and cross-core reduced with a single
    collective (ReduceScatter(add), or AllToAll + local adds) over
    batch, so core c ends up with the fully reduced batch columns
    [64c, 64c+64).  A tiny dummy collective issued at program start
    pulls the one-time CC bootstrap barrier off the critical path.
  - Each core applies bias + tanh/sigmoid gating and the small W2
    GEMM for its batch chunk, writing logits.T [V, 64].
  - Host assembles/transposes the 8 chunks back to [B, V].
"""

import os

import numpy as np

import concourse.bacc as bacc
import concourse.bass as bass
import concourse.mybir as mybir
import concourse.tile as tile
from concourse import bass_utils

B, M, V, H = 512, 128, 256, 256
IN = M * V              # 32768
NCORES = 8
KC = IN // NCORES       # 4096 contraction rows per core
NKG = 8                 # DMA k-groups per core
KB_PER_G = KC // (NKG * 128)  # 4 k-chunks of 128 per group
BCHUNK = B // NCORES    # 64 batch columns per core after reduce-scatter

F32 = mybir.dt.float32
F32R = mybir.dt.float32r
BF16 = mybir.dt.bfloat16
AF = mybir.ActivationFunctionType

DUMMY = os.environ.get("KERNEL_DUMMY", "quad")  # quad | full | none
CC = os.environ.get("KERNEL_CC", "rs")  # rs | a2a

_CACHE = {}


def _stage2(nc, s2pool, psum2, s2, bt, w2t, outT):
    """Gate + W2 for the local batch chunk.

    s2: SBUF AP [128, 4, BCHUNK] holding [p1_h0, p1_h1, pg_h0, pg_h1]
    row-blocks of the fully-reduced partials for this core's chunk.
    """
    hT = []
    for i in range(2):
        th = s2pool.tile([128, BCHUNK], F32, tag=f"th{i}", name=f"th{i}")
        nc.scalar.activation(th[:], s2[:, i, :], AF.Tanh, bias=bt[:, i : i + 1])
        sg = s2pool.tile([128, BCHUNK], F32, tag=f"sg{i}", name=f"sg{i}")
        nc.scalar.activation(
            sg[:], s2[:, 2 + i, :], AF.Sigmoid, bias=bt[:, 2 + i : 3 + i]
        )
        ht = s2pool.tile([128, BCHUNK], BF16, tag=f"ht{i}", name=f"ht{i}")
        nc.vector.tensor_mul(ht[:], th[:], sg[:])
        hT.append(ht)

    ot = s2pool.tile([128, 2, BCHUNK], F32, tag="ot", name="ot")
    for v in range(2):
        ps = psum2.tile([128, BCHUNK], F32, tag=f"acc{v}", name=f"ps2_{v}")
        for i in range(2):
            nc.tensor.matmul(
                ps[:],
                w2t[:, i, bass.ts(v, 128)],
                hT[i][:],
                start=(i == 0),
                stop=(i == 1),
            )
        nc.vector.tensor_scalar_add(ot[:, v, :], ps[:], bt[:, 4 + v : 5 + v])
    nc.sync.dma_start(out=outT.rearrange("(v p) b -> p v b", p=128), in_=ot[:])


def _build(dummy=DUMMY, cc=CC):
    nc = bacc.Bacc(
        "TRN2",
        target_bir_lowering=False,
        debug=False,
        enable_asserts=False,
        num_devices=NCORES,
    )

    GROUPS = [list(range(NCORES))]

    # Per-core external inputs (host pre-packed so each DMA moves long
    # contiguous lines per partition: memT 4KB, weights 2KB).
    memT = nc.dram_tensor("memT", [NKG, 128, KB_PER_G * B], BF16, kind="ExternalInput")
    w1 = nc.dram_tensor("w1", [NKG, 128, KB_PER_G * H], BF16, kind="ExternalInput")
    wg = nc.dram_tensor("wg", [NKG, 128, KB_PER_G * H], BF16, kind="ExternalInput")
    # W2 pre-transposed to [p, c, v] bf16 on host.
    w2 = nc.dram_tensor("w2", [128, 2, V], BF16, kind="ExternalInput")
    # packed biases: cols = [b1_lo, b1_hi, bg_lo, bg_hi, b2_lo, b2_hi]
    bpk = nc.dram_tensor("bpk", [128, 6], F32, kind="ExternalInput")
    outT = nc.dram_tensor("outT", [V, BCHUNK], F32, kind="ExternalOutput")

    with tile.TileContext(nc) as tc:
        with (
            tc.tile_pool(name="xg", bufs=4) as xpool,
            tc.tile_pool(name="wt", bufs=4) as wpool,
            tc.tile_pool(name="part", bufs=1) as ppool,
            tc.tile_pool(name="s2", bufs=1) as s2pool,
            tc.tile_pool(name="psum1", bufs=1, space="PSUM") as psum1,
            tc.tile_pool(name="dram", bufs=1, space="DRAM") as dpool,
        ):
            # Tiny dummy collective issued first: the one-time CC
            # bootstrap barrier completes early (it waits on every
            # core's first stream trigger), so the real collective is
            # not stalled behind it.
            if dummy != "none":
                dgroups = (
                    [[0, 1, 2, 3], [4, 5, 6, 7]] if dummy == "quad" else GROUPS
                )
                nsub = len(dgroups[0])
                dd_in = dpool.tile([nsub, 64], BF16, tag="ddin", name="ddin")
                dd_out = dpool.tile([64], BF16, tag="ddout", name="ddout")
                dseed = s2pool.tile([nsub, 64], BF16, tag="dseed")
                nc.gpsimd.memset(dseed[:], 0.0)
                nc.gpsimd.dma_start(out=dd_in[:], in_=dseed[:])
                nc.gpsimd.collective_compute(
                    "ReduceScatter",
                    mybir.AluOpType.add,
                    replica_groups=dgroups,
                    ins=[dd_in[:].opt()],
                    outs=[dd_out[:].opt()],
                )

            # Pre-warm the Tanh/Sigmoid activation tables off the critical
            # path (the first use of each table pays a ~1.3us load).
            warm = s2pool.tile([128, 1], F32, tag="warm")
            nc.gpsimd.memset(warm[:], 0.0)
            warm2 = s2pool.tile([128, 1], F32, tag="warm2")
            nc.scalar.activation(warm2[:], warm[:], AF.Tanh)
            nc.scalar.activation(warm[:], warm2[:], AF.Sigmoid)

            # Pre-warm the PE HAM clock gate with dummy matmuls while the
            # first input DMAs are in flight (~4us of sustained PE activity
            # releases the 4/8 throttle).
            wsrc = s2pool.tile([128, B], BF16, tag="wsrc")
            nc.gpsimd.memset(wsrc[:], 0.0)
            wps = psum1.tile([128, B], F32, tag="acc7", name="wps")
            for i in range(20):
                nc.tensor.matmul(
                    wps[:], wsrc[:, 0:128], wsrc[:], start=(i == 0), stop=(i == 19)
                )

            # Stage-2 constants on the (otherwise idle) gpsimd SWDGE queue.
            bt = s2pool.tile([128, 6], F32, tag="bias")
            nc.gpsimd.dma_start(out=bt[:], in_=bpk[:, :])
            w2t = s2pool.tile([128, 2, V], BF16, tag="w2")
            nc.gpsimd.dma_start(out=w2t[:], in_=w2[:, :, :])

            # ---------------- stage 1: partial W.T @ flat.T ----------------
            acc = [
                psum1.tile([128, B], F32, tag=f"acc{t}", name=f"acc_{t}")
                for t in range(4)
            ]

            # ccin chunk layout [c][p][t*b]: core c's received chunk reads
            # back contiguously per partition.
            ccin = dpool.tile(
                [NCORES, 128, 4 * BCHUNK], BF16, tag="ccin", name="ccin"
            )

            NKB = NKG * KB_PER_G  # 32 k-chunks of 128
            for kg in range(NKG):
                xg = xpool.tile([128, KB_PER_G * B], BF16, tag="xg")
                nc.sync.dma_start(out=xg[:], in_=memT[kg])
                w1t = wpool.tile([128, KB_PER_G * H], BF16, tag="w1t")
                nc.scalar.dma_start(out=w1t[:], in_=w1[kg])
                wgt = wpool.tile([128, KB_PER_G * H], BF16, tag="wgt")
                nc.scalar.dma_start(out=wgt[:], in_=wg[kg])
                for kb in range(KB_PER_G):
                    k = kg * KB_PER_G + kb
                    first = k == 0
                    last = k == NKB - 1
                    rhs = xg[:, bass.ts(kb, B)]
                    for h in range(2):
                        nc.tensor.matmul(
                            acc[h][:],
                            w1t[:, bass.ts(2 * kb + h, 128)],
                            rhs,
                            start=first,
                            stop=last,
                        )
                        nc.tensor.matmul(
                            acc[2 + h][:],
                            wgt[:, bass.ts(2 * kb + h, 128)],
                            rhs,
                            start=first,
                            stop=last,
                        )

            # PSUM -> SBUF (f32 -> bf16 downcast) -> DRAM scatter layout.
            for t in range(4):
                sbt = ppool.tile([128, B], BF16, tag=f"po{t}", name=f"po_{t}")
                nc.vector.tensor_copy(sbt[:], acc[t][:])
                nc.gpsimd.dma_start(
                    out=ccin[:, :, bass.ts(t, BCHUNK)].rearrange("c p b -> p c b"),
                    in_=sbt[:].rearrange("p (c b) -> p c b", c=NCORES),
                )

            if cc == "rs":
                ccout = dpool.tile([128, 4 * BCHUNK], BF16, tag="ccout", name="ccout")
                nc.gpsimd.collective_compute(
                    "ReduceScatter",
                    mybir.AluOpType.add,
                    replica_groups=GROUPS,
                    ins=[ccin[:].opt()],
                    outs=[ccout[:].opt()],
                )
                s2 = s2pool.tile([128, 4, BCHUNK], BF16, tag="s2in")
                nc.sync.dma_start(
                    out=s2[:], in_=ccout.rearrange("p (t b) -> p t b", t=4)
                )
            else:
                cc2 = dpool.tile(
                    [NCORES, 128, 4 * BCHUNK], BF16, tag="cc2", name="cc2"
                )
                nc.gpsimd.collective_compute(
                    "AllToAll",
                    mybir.AluOpType.bypass,
                    replica_groups=GROUPS,
                    ins=[ccin[:].opt()],
                    outs=[cc2[:].opt()],
                )
                rr = s2pool.tile([128, NCORES, 4, BCHUNK], BF16, tag="rr")
                nc.sync.dma_start(
                    out=rr[:],
                    in_=cc2.rearrange("c p (t b) -> p c t b", t=4),
                )
                a1 = s2pool.tile([128, 4, 4, BCHUNK], BF16, tag="a1")
                nc.vector.tensor_add(a1[:], rr[:, 0:4, :, :], rr[:, 4:8, :, :])
                a2 = s2pool.tile([128, 2, 4, BCHUNK], BF16, tag="a2")
                nc.vector.tensor_add(a2[:], a1[:, 0:2, :, :], a1[:, 2:4, :, :])
                s2 = s2pool.tile([128, 4, BCHUNK], BF16, tag="s2in")
                nc.vector.tensor_add(s2[:], a2[:, 0, :, :], a2[:, 1, :, :])

            # ---------------- stage 2: gate + W2 ----------------
            _stage2(nc, s2pool, psum1, s2, bt, w2t, outT)

    nc.compile()
    return nc


def _shard(x, memory, W1, b1, Wg, bg, W2, b2):
    """Build the 8 per-core input maps from the full problem inputs."""
    import ml_dtypes

    bf16 = ml_dtypes.bfloat16
    x = np.asarray(x, dtype=np.float32)
    memory = np.asarray(memory, dtype=np.float32)
    W1 = np.asarray(W1, dtype=np.float32)
    Wg = np.asarray(Wg, dtype=np.float32)
    W2 = np.ascontiguousarray(np.asarray(W2, dtype=np.float32))
    b1 = np.asarray(b1, dtype=np.float32)
    bg = np.asarray(bg, dtype=np.float32)
    b2 = np.asarray(b2, dtype=np.float32)

    # rolled ring buffer, flattened and transposed: [IN, B]
    flatT = np.empty((IN, B), dtype=np.float32)
    flatT[:V] = x.T
    flatT[V:] = memory[:, : M - 1, :].reshape(B, IN - V).T
    bpk = np.ascontiguousarray(
        np.stack([b1[:128], b1[128:], bg[:128], bg[128:], b2[:128], b2[128:]], axis=1)
    )
    w2p = np.ascontiguousarray(
        W2.reshape(2, 128, V).transpose(1, 0, 2).astype(bf16)
    )

    def pack(A):
        # [KC, X] -> [NKG, 128, KB_PER_G * X]: per-partition lines are
        # KB_PER_G consecutive k-chunks' rows, contiguous in DRAM.
        X = A.shape[1]
        return np.ascontiguousarray(
            A.reshape(NKG, KB_PER_G, 128, X)
            .transpose(0, 2, 1, 3)
            .reshape(NKG, 128, KB_PER_G * X)
        )

    in_maps = []
    for c in range(NCORES):
        sl = slice(KC * c, KC * (c + 1))
        in_maps.append(
            {
                "memT": pack(flatT[sl].astype(bf16)),
                "w1": pack(W1[sl].astype(bf16)),
                "wg": pack(Wg[sl].astype(bf16)),
                "w2": w2p,
                "bpk": bpk,
            }
        )
    return in_maps


def _get_nc():
    if "nc" not in _CACHE:
        _CACHE["nc"] = _build()
    return _CACHE["nc"]


def kernel(x, memory, W1, b1, Wg, bg, W2, b2, **run_kwargs):
    nc = _get_nc()
    in_maps = _shard(x, memory, W1, b1, Wg, bg, W2, b2)
    res = bass_utils.run_bass_kernel_spmd(
        nc, in_maps, core_ids=list(range(NCORES)), **run_kwargs
    )
    _CACHE["last_results"] = res
    out = np.empty((B, V), dtype=np.float32)
    for c in range(NCORES):
        out[c * BCHUNK : (c + 1) * BCHUNK, :] = res.results[c]["outT"].T
    return out
